# revision 24
# baseline (speedup 1.0000x reference)
"""MoE + LoRA expert FFN kernel for 8 Trainium2 NeuronCores.

Strategy (expert-parallel, host dispatch/combine):
  - E=8 experts, one expert per core. The host groups tokens by expert
    (a token appears once per distinct selected expert; duplicate
    selections collapse with summed routing weight), pads each group to
    a uniform capacity C (= max group size), and ships per-core inputs:
        xT   [H, C]      tokens routed to this core's expert, transposed
        wgpA/wgpB        gate_proj + 2*gate_A@gate_B, packed into slabs
        wupA/wupB        up_proj   + 2*up_A@up_B,     packed into slabs
        wd   [I, H]      down_proj + 2*down_A@down_B
    and receives yT [H, C] bf16 = (silu(x@wg) * (x@wu)) @ wd, transposed.
  - All matmul operands are bf16 (PE runs bf16 at 1 col/cycle and FWL
    engages for the weight loads; fp32r is the same mult speed but has
    slow non-FWL LDWEIGHTS and 2x the DMA bytes). PSUM accumulation is
    fp32; measured end-to-end relative error ~5e-3 vs the 2e-2 gate.
  - wg/wu are packed on the host into PE-consumption-ordered slabs, one
    contiguous 256/512KB DMA per (column group, projection). The HWDGE
    trigger instruction costs ~0.7us on the issuing engine, so small
    per-tile DMAs would cap a ring at ~50GB/s. Column groups ramp
    1,1,2,.. so the first mults need only x + 0.5MB of weights (the DMA
    engines only start moving ~9us in; the critical mass sets the PE
    start time, covered by warmup dummy matmuls that also hold the HAM
    at full array width).
  - The host scales each token's expert output by its routing weight and
    scatters back into the [T, H] result.

LoRA folding is exact algebra: x@W + s*(x@A)@B == x@(W + s*A@B).
"""

import numpy as np
import ml_dtypes

E, H, I, R, TOPK = 8, 1024, 2816, 8, 2
SCALING = 2.0
NCORES = 8
KP = 128          # partition / contraction tile
NTOK = 512        # moving-dim (token) tile
KH = H // KP      # 8 contraction chunks over H
KI = I // KP      # 22 chunks over I
GROUPS = [1, 1] + [2] * ((KI - 2) // 2)   # i-tiles per weight column group
BF16 = ml_dtypes.bfloat16

_cache = {}


def _setup_paths():
    import sys
    for p in ("/opt/trn_rl_repo", "/root/.axon_site"):
        if p not in sys.path:
            sys.path.insert(0, p)


def _split_multi_waits(nc):
    """The walrus in this container accepts at most 1 sem wait per
    instruction (2 on EventSemaphore); Tile emits more. Rewrite each block,
    moving excess waits onto preceding single-wait NoOps on the same
    engine (engines execute in order, so semantics are preserved)."""
    _setup_paths()
    from bass_rust import SyncInfo
    from concourse import mybir

    ctr = [0]
    for f in nc.m.functions:
        for bb in f.blocks:
            insts = bb.instructions
            new = []
            changed = False
            for inst in insts:
                si = inst.sync_info
                waits = list(si.on_wait or []) if si is not None else []
                cap = 2 if isinstance(inst, mybir.InstEventSemaphore) else 1
                if len(waits) > cap:
                    changed = True
                    for w in waits[:-cap]:
                        nop = mybir.InstNoOp(
                            name=f"SW-{ctr[0]}", ins=[], outs=[])
                        ctr[0] += 1
                        nop.engine = inst.engine
                        nop.sync_info = SyncInfo(on_wait=[w], on_update=[])
                        new.append(nop)
                    inst.sync_info = SyncInfo(
                        on_wait=waits[-cap:],
                        on_update=list(si.on_update or []))
                new.append(inst)
            if changed:
                bb.instructions = new


def _token_tiles(C):
    tiles = []
    t0 = 0
    while t0 < C:
        tw = min(NTOK, C - t0)
        tiles.append((t0, tw))
        t0 += tw
    return tiles


def _build(C):
    """Build the per-core Bass program for token capacity C."""
    _setup_paths()
    import concourse.bass as bass
    import concourse.tile as tile
    from concourse import mybir

    f32 = mybir.dt.float32
    bf = mybir.dt.bfloat16

    HH = H // KP            # 8 output row blocks
    NG = len(GROUPS)
    gstart = [sum(GROUPS[:j]) for j in range(NG)]
    i2q = {}
    for q, (g0, gn) in enumerate(zip(gstart, GROUPS)):
        for i in range(g0, g0 + gn):
            i2q[i] = (q, i - g0)
    NGA = sum(1 for g in GROUPS if g == 1)   # leading 1-wide groups
    NGB = NG - NGA

    nc = bass.Bass("TRN2", target_bir_lowering=False, debug=False,
                   num_devices=NCORES)
    xT = nc.declare_dram_parameter("xT", [H, C], bf, isOutput=False)
    # weight slabs: one contiguous DMA per (group, projection). The HWDGE
    # trigger instruction costs ~0.7us on the issuing engine, so small
    # per-tile DMAs cap a ring at ~50GB/s — slabs restore full rate.
    wgpA = nc.declare_dram_parameter("wgpA", [NGA * KP, KH * KP], bf,
                                     isOutput=False)
    wupA = nc.declare_dram_parameter("wupA", [NGA * KP, KH * KP], bf,
                                     isOutput=False)
    wgpB = nc.declare_dram_parameter("wgpB", [NGB * KP, KH * 2 * KP], bf,
                                     isOutput=False)
    wupB = nc.declare_dram_parameter("wupB", [NGB * KP, KH * 2 * KP], bf,
                                     isOutput=False)
    wd = nc.declare_dram_parameter("wd", [I, H], bf, isOutput=False)
    yT = nc.declare_dram_parameter("yT", [H, C], bf, isOutput=True)

    ttiles = _token_tiles(C)

    with tile.TileContext(nc) as tc:
        # single PSUM pool spanning both phases: phase D reuses phase B's
        # g-tags, so its first matmuls recycle long-drained slots instead
        # of waiting on a pool-close barrier
        with tc.tile_pool(name="hh", bufs=1) as hp, \
             tc.tile_pool(name="wdp", bufs=1) as wdp, \
             tc.tile_pool(name="ps", bufs=2, space="PSUM") as ps:
            h_t = [hp.tile([KP, C], bf, tag=f"h{i}", name=f"h{i}")
                   for i in range(KI)]

            # wd tiles live in an outer pool; loads are issued throughout
            # phase B on the gpsimd (SWDGE) ring — none up-front, so they
            # stay out of the startup DMA critical mass — and all of wd is
            # resident well before phase D needs it.
            wd_t = {}

            def load_wd(i):
                t = wdp.tile([KP, H], bf, tag=f"wds{i}", name=f"wds{i}")
                nc.gpsimd.dma_start(out=t, in_=wd[i * KP:(i + 1) * KP, :])
                wd_t[i] = t

            # ---- phase B: h = silu(x@wg) * (x@wu), feature-major [I, C]
            with tc.tile_pool(name="xp", bufs=1) as xp, \
                 tc.tile_pool(name="wst", bufs=2) as wst, \
                 tc.tile_pool(name="actB", bufs=4) as actB:
                # The startup is DMA-critical-mass bound: the DMA engines
                # only start moving ~9us in (preamble + trigger latency).
                # Whole 256KB x tiles are spread across all three rings in
                # consumption order; weight slabs are one DMA per (group,
                # projection) so no ring is trigger-rate limited.
                x_t = [xp.tile([KP, C], bf, tag=f"x{k}", name=f"x{k}")
                       for k in range(KH)]

                def ld_x(k, eng):
                    eng.dma_start(out=x_t[k],
                                  in_=xT[k * KP:(k + 1) * KP, :])

                wg_s, wu_s = {}, {}

                def load_w_slab(q, proj):
                    cw = GROUPS[q] * KP
                    w = KH * cw
                    if q < NGA:
                        src = (wgpA, wupA)[proj]
                        r0 = q * KP
                        tag = ("wgA", "wuA")[proj]
                        shape = [KP, KH * KP]
                    else:
                        src = (wgpB, wupB)[proj]
                        r0 = (q - NGA) * KP
                        tag = ("wgB", "wuB")[proj]
                        shape = [KP, KH * 2 * KP]
                    t = wst.tile(shape, bf, tag=tag,
                                 name=f"w{'gu'[proj]}_s{q}", bufs=2 if
                                 q < NGA else 4)
                    eng = (nc.sync, nc.scalar)[proj]
                    if q < NGA:
                        # first groups: two half-DMAs so the earliest
                        # mults wait on 128KB, not the whole slab
                        h = w // 2
                        eng.dma_start(out=t[:, :h], in_=src[r0:r0 + KP, :h])
                        eng.dma_start(out=t[:, h:w],
                                      in_=src[r0:r0 + KP, h:w])
                    else:
                        eng.dma_start(out=t[:, :w],
                                      in_=src[r0:r0 + KP, :w])
                    if proj == 0:
                        wg_s[q] = t
                    else:
                        wu_s[q] = t

                # consumption-ordered emission per ring
                ld_x(0, nc.gpsimd)
                ld_x(1, nc.sync)
                ld_x(2, nc.scalar)
                load_w_slab(0, 0)                   # wg q0 (sync)
                load_w_slab(0, 1)                   # wu q0 (scalar)
                ld_x(3, nc.gpsimd)
                ld_x(4, nc.sync)
                ld_x(5, nc.scalar)
                load_w_slab(1, 0)                   # wg q1 (sync)
                load_w_slab(1, 1)                   # wu q1 (scalar)
                ld_x(6, nc.gpsimd)
                ld_x(7, nc.gpsimd)

                def load_w_group(q):
                    load_w_slab(q, 0)
                    load_w_slab(q, 1)

                # short warmup: the PE HAM un-throttles while the first
                # DMAs land; real mults start ~12us and are DMA-gated (at
                # half clock until the HAM grant, which also lets the DMA
                # catch up)
                wsrc = actB.tile([KP, 256], bf, tag="wsrc", name="wsrc")
                nc.vector.memset(wsrc, 0.0)
                wdst = ps.tile([KP, NTOK], f32, tag="g0", name="wdst",
                                padded_shape=[KP, NTOK])
                for w in range(40):
                    nc.tensor.matmul(wdst[:, :256], wsrc[:, :128], wsrc,
                                     start=(w == 0), stop=(w == 39))

                # PSUM: 4 tag families (g/u x token tile), bufs=2 each =
                # exactly 8 banks at NT=2. Stationary weights are reused
                # across the NT token tiles (one LDWEIGHTS per k-chunk).
                def gu_mults(i, tis, fill=None):
                    q, r = i2q[i]
                    cw = GROUPS[q] * KP
                    wsl = {k: slice(k * cw + r * KP,
                                    k * cw + (r + 1) * KP)
                           for k in range(KH)}
                    g_ps = {ti: ps.tile([KP, ttiles[ti][1]], f32,
                                         tag=f"g{ti}",
                                         name=f"g{i}_{ttiles[ti][0]}",
                                         padded_shape=[KP, NTOK])
                            for ti in tis}
                    u_ps = {ti: ps.tile([KP, ttiles[ti][1]], f32,
                                         tag=f"u{ti}",
                                         name=f"u{i}_{ttiles[ti][0]}",
                                         padded_shape=[KP, NTOK])
                            for ti in tis}
                    for k in range(KH):
                        for ti in tis:
                            t0, tw = ttiles[ti]
                            nc.tensor.matmul(
                                g_ps[ti], wg_s[q][:, wsl[k]],
                                x_t[k][:, t0:t0 + tw],
                                start=(k == 0), stop=(k == KH - 1))
                        if fill is not None and k % 2 == 1:
                            nc.tensor.matmul(fill[:, :256], wsrc[:, :128],
                                             wsrc, start=True, stop=True)
                    for k in range(KH):
                        for ti in tis:
                            t0, tw = ttiles[ti]
                            nc.tensor.matmul(
                                u_ps[ti], wu_s[q][:, wsl[k]],
                                x_t[k][:, t0:t0 + tw],
                                start=(k == 0), stop=(k == KH - 1))
                        if fill is not None and k % 2 == 1:
                            nc.tensor.matmul(fill[:, :256], wsrc[:, :128],
                                             wsrc, start=True, stop=True)
                    for ti in tis:
                        t0, tw = ttiles[ti]
                        sg = actB.tile([KP, tw], f32, tag=f"sg{ti}",
                                       name=f"sg{i}_{t0}")
                        nc.scalar.activation(
                            sg, g_ps[ti], mybir.ActivationFunctionType.Silu)
                        nc.vector.tensor_mul(
                            h_t[i][:, t0:t0 + tw], sg, u_ps[ti])

                NTt = len(ttiles)
                # unpaired warm-start: token half 0 of i=0,1 only needs
                # x's first 1MB; half 1 runs once the rest of x lands.
                # Groups 2 and 3 load behind the warm-start passes (the
                # JIT loop below only triggers from group 4 on).
                gu_mults(0, [0])
                load_w_group(2)
                gu_mults(1, [0])
                load_w_group(3)
                for i in (0, 1):
                    gu_mults(i, list(range(1, NTt)))
                for i in range(2, KI):
                    q, r = i2q[i]
                    if r == 0 and q + 2 < NG:
                        load_w_group(q + 2)
                    # trickle wd loads once the startup DMA crunch is over
                    if i >= 3:
                        if (i - 3) not in wd_t:
                            load_wd(i - 3)
                        if i == KI - 1:
                            for j in range(KI):
                                if j not in wd_t:
                                    load_wd(j)
                    gu_mults(i, list(range(NTt)))

            # ---- phase D: yT = h @ wd, output [H, C] bf16
            # hh-outer, i inner, token tiles paired per stationary load.
            # All wd tiles are already resident; each output block finishes
            # ~KI*NT*213ns apart so copies/stores are fully staggered. The
            # store is split across the two idle HWDGE rings so the last
            # block drains in ~0.7us.
            with tc.tile_pool(name="yout", bufs=3) as yp:
                for hh in range(HH):
                    y_ps = [ps.tile([KP, tw], f32, tag=f"g{ti}",
                                    name=f"y{hh}_{t0}",
                                    padded_shape=[KP, NTOK])
                            for ti, (t0, tw) in enumerate(ttiles)]
                    for i in range(KI):
                        for ti, (t0, tw) in enumerate(ttiles):
                            nc.tensor.matmul(
                                y_ps[ti],
                                wd_t[i][:, hh * KP:(hh + 1) * KP],
                                h_t[i][:, t0:t0 + tw],
                                start=(i == 0), stop=(i == KI - 1))
                    # copies alternate engines (vector/scalar) so they
                    # overlap; each ti's store rides its own HWDGE ring
                    # (the store is latency-bound at ~2us regardless of
                    # size, so finer chunking doesn't pay)
                    yo = yp.tile([KP, C], bf, tag="yo", name=f"yo{hh}")
                    rings = [nc.sync, nc.scalar]
                    for ti, (t0, tw) in enumerate(ttiles):
                        if ti % 2 == 0:
                            nc.vector.tensor_copy(yo[:, t0:t0 + tw],
                                                  y_ps[ti])
                        else:
                            nc.scalar.activation(
                                yo[:, t0:t0 + tw], y_ps[ti],
                                mybir.ActivationFunctionType.Copy)
                        rings[ti % 2].dma_start(
                            out=yT[hh * KP:(hh + 1) * KP, t0:t0 + tw],
                            in_=yo[:, t0:t0 + tw])
    _split_multi_waits(nc)
    return nc


CMAX = 1024   # per-run token capacity (bounded by SBUF for the h tiles)


def _pack_w(w):
    """[H, I] -> per-group slabs: A [NGA*128, KH*128] for the 1-wide
    groups, B [NGB*128, KH*256] for the 2-wide groups. Slab row p holds
    w[k*128+p, cols] for the KH contraction chunks side by side, so each
    slab is one contiguous DMA in PE-consumption order."""
    NG = len(GROUPS)
    gstart = [sum(GROUPS[:j]) for j in range(NG)]
    NGA = sum(1 for g in GROUPS if g == 1)
    outA = np.zeros((NGA * KP, KH * KP), dtype=BF16)
    outB = np.zeros(((NG - NGA) * KP, KH * 2 * KP), dtype=BF16)
    for q in range(NG):
        c0 = gstart[q] * KP
        cw = GROUPS[q] * KP
        blk = w[:, c0:c0 + cw]                    # [H, cw]
        blk = blk.reshape(KH, KP, cw).transpose(1, 0, 2).reshape(
            KP, KH * cw)                          # [128, KH*cw]
        if q < NGA:
            outA[q * KP:(q + 1) * KP] = blk
        else:
            outB[(q - NGA) * KP:(q - NGA + 1) * KP] = blk
    return outA, outB


def _prepare(inputs):
    """Host-side routing + weight folding. Returns per-core tensors."""
    hs = np.asarray(inputs["hidden_states"], dtype=np.float32)
    rw = np.asarray(inputs["routing_weights"], dtype=np.float32)
    se = np.asarray(inputs["selected_experts"]).astype(np.int64)
    T = hs.shape[0]

    combine = np.zeros((T, E), dtype=np.float32)
    for k in range(se.shape[1]):
        np.add.at(combine, (np.arange(T), se[:, k]), rw[:, k])

    idx = [np.nonzero(combine[:, e])[0] for e in range(E)]
    wts = [combine[idx[e], e] for e in range(E)]
    maxn = max((len(ix) for ix in idx), default=1)
    C = min(max(KP, maxn), CMAX)

    gp = np.asarray(inputs["gate_proj"], dtype=np.float32)
    up = np.asarray(inputs["up_proj"], dtype=np.float32)
    dp = np.asarray(inputs["down_proj"], dtype=np.float32)
    gA = np.asarray(inputs["gate_A"], dtype=np.float32)
    gB = np.asarray(inputs["gate_B"], dtype=np.float32)
    uA = np.asarray(inputs["up_A"], dtype=np.float32)
    uB = np.asarray(inputs["up_B"], dtype=np.float32)
    dA = np.asarray(inputs["down_A"], dtype=np.float32)
    dB = np.asarray(inputs["down_B"], dtype=np.float32)

    wmaps = []
    for e in range(E):
        wge = gp[e] + SCALING * (gA[e] @ gB[e])
        wue = up[e] + SCALING * (uA[e] @ uB[e])
        wde = (dp[e] + SCALING * (dA[e] @ dB[e])).astype(BF16)
        gpA, gpB = _pack_w(wge)
        upA, upB = _pack_w(wue)
        wmaps.append({"wgpA": gpA, "wgpB": gpB,
                      "wupA": upA, "wupB": upB, "wd": wde})
    return hs, wmaps, idx, wts, C


def kernel(**inputs):
    _setup_paths()
    from concourse.bass_utils import run_bass_kernel_spmd

    hs, wmaps, idx, wts, C = _prepare(inputs)

    nc = _cache.get(C)
    if nc is None:
        nc = _build(C)
        _cache[C] = nc

    T = hs.shape[0]
    out = np.zeros((T, H), dtype=np.float32)
    maxn = max((len(ix) for ix in idx), default=1)
    nruns = max(1, -(-maxn // C))
    for r in range(nruns):
        in_maps = []
        for e in range(E):
            sub = idx[e][r * C:(r + 1) * C]
            xTe = np.zeros((H, C), dtype=BF16)
            if len(sub):
                xTe[:, :len(sub)] = hs[sub].T.astype(BF16)
            in_maps.append({"xT": xTe, **wmaps[e]})
        try:
            res = run_bass_kernel_spmd(
                nc, in_maps, core_ids=list(range(NCORES)))
        except Exception:
            import time
            time.sleep(2.0)
            res = run_bass_kernel_spmd(
                nc, in_maps, core_ids=list(range(NCORES)))

        # expose for external profiling harnesses (test.py)
        kernel._last = {"nc": nc, "in_maps": in_maps, "results": res}

        for e in range(E):
            sub = idx[e][r * C:(r + 1) * C]
            if not len(sub):
                continue
            w = wts[e][r * C:(r + 1) * C]
            yTe = res.results[e]["yT"]          # [H, C] bf16
            out[sub] += w[:, None] * yTe[:, :len(sub)].T.astype(np.float32)
    return out
